# revision 16
# baseline (speedup 1.0000x reference)
"""Self-contained Trainium2 Bass kernel for sliding-window attention.

Problem (hardcoded): B=1, S=8192, dim=1024, H=16 heads, D=64 head dim,
window=512, fp32 I/O.  y = (softmax(mask(rope(xWq^T) rope(xWk^T)^T / 8)) xWv^T) Wo^T

Strategy: sequence-parallel over 8 NeuronCores. Each core owns 1024 query
rows and additionally recomputes K/V for the 512-row halo to its left
(core 0's halo is zero-padded and neutralized via a per-core "vones"
column so no collective is needed).  All matmuls run in bf16 (fp32 PSUM
accumulation); sliding-window causality is applied by extra rank-128
mask matmuls accumulated into the score PSUM before the exp.

Layouts (per core):
  xT    [1024(d), 1536(s)]  x^T shard incl. halo (bf16)
  wq/wk [1024(d), 1024(e')] Wq^T / Wk^T with a per-head even/odd column
                            permutation (rope pair de-interleave: head h's
                            rows are [evens(32) | odds(32)])
  wv    [1024(d), 1024(e)]  Wv^T (no permutation), wo = Wo^T
  Q^T/K^T are produced in [e', s] layout (weight-stationary matmuls) so
  attention needs no transposes: scores are computed transposed,
  S^T[k, q], the softmax denominator comes free from a ones-column
  appended to V, and PV directly yields o^T[e, q] — the lhsT of the
  output projection.  Rope runs on the vector engine using 32-periodic
  cos/sin tables (equal-base operand reads + partition-shifted writes).
"""
import sys

sys.path.insert(0, "/opt/trn_rl_repo")

import numpy as np
import ml_dtypes

import concourse.bass as bass
import concourse.mybir as mybir
from concourse import bacc
from concourse.tile import TileContext
from concourse.bass_utils import run_bass_kernel_spmd

BF = ml_dtypes.bfloat16
NCORES = 8
S, DIM, H, D, W = 8192, 1024, 16, 64, 512
SL = S // NCORES          # 1024 own rows / core
SK = SL + W               # 1536 rows incl. left halo
P = 128
NKT = SK // P             # 12 kv tiles
NQB = SL // P             # 8 query tiles
dt = mybir.dt

_compiled = {}


def _build(dbg=False):
    nc = bacc.Bacc("TRN2", target_bir_lowering=False, debug=False,
                   num_devices=NCORES)
    def param(name, shape, dtype=dt.bfloat16, out=False):
        return nc.declare_dram_parameter(name, shape, dtype, isOutput=out)

    xt = param("xt", [DIM, SK])
    wq = param("wq", [DIM, DIM])
    wk = param("wk", [DIM, DIM])
    wv = param("wv", [DIM, DIM])
    wo = param("wo", [DIM, DIM])
    ropc = param("ropc", [P, SK])
    rops = param("rops", [P, SK])
    vone = param("vone", [SK, 1])
    uold = param("uold", [P, P])
    udia = param("udia", [P, P])
    negi = param("negi", [P, P])
    out = param("out", [SL, DIM], dt.float32, out=True)
    dbg_outs = None
    if dbg:
        dbg_outs = {
            "d_qt": param("d_qt", [P, 8 * SL], out=True),
            "d_kt": param("d_kt", [P, 8 * SK], out=True),
            "d_v": param("d_v", [P, NKT * H * 80], out=True),
            "d_ot": param("d_ot", [P, 8 * SL], out=True),
                                }

    with TileContext(nc) as tc:
        _body(nc, tc, xt, wq, wk, wv, wo, ropc, rops, vone, uold, udia,
              negi, out, dbg_outs)
    nc.compile()
    return nc


def _brd2(ap_slice, n):
    """Insert a stride-0 middle free dim of size n into a [p, c] AP."""
    return bass.AP(tensor=ap_slice.tensor, offset=ap_slice.offset,
                   ap=[ap_slice.ap[0], [0, n], ap_slice.ap[1]])


VA = 80   # V_aug columns: 64 V + 1 ones + pad (32B-aligned stride)


def _body(nc, tc, xt, wq, wk, wv, wo, ropc, rops, vone, uold, udia, negi,
          out, dbg_outs=None):
    f32, bf16 = dt.float32, dt.bfloat16
    mult, add = mybir.AluOpType.mult, mybir.AluOpType.add

    with tc.tile_pool(name="persist", bufs=1) as per:
        # long-lived SBUF tensors
        v_sb = per.tile([P, NKT, H, VA], bf16)    # V_aug: [V(64)|ones|pad]
        qt_sb = per.tile([P, 8, SL], bf16)        # Q^T (rope'd, sigma layout)
        kt_sb = per.tile([P, 8, SK], bf16)        # K^T
        ot_sb = per.tile([P, 8, SL], bf16)        # o^T (normalized)
        ropc_sb = per.tile([P, SK], bf16)
        rops_sb = per.tile([P, SK], bf16)
        uold_sb = per.tile([P, P], bf16)
        udia_sb = per.tile([P, P], bf16)
        negi_sb = per.tile([P, P], bf16)
        vone_sb = per.tile([P, NKT], bf16)

        nc.sync.dma_start(out=ropc_sb, in_=ropc[:, :])
        nc.sync.dma_start(out=rops_sb, in_=rops[:, :])
        nc.sync.dma_start(out=uold_sb, in_=uold[:, :])
        nc.sync.dma_start(out=udia_sb, in_=udia[:, :])
        nc.sync.dma_start(out=negi_sb, in_=negi[:, :])
        nc.sync.dma_start(out=vone_sb,
                          in_=vone.ap().rearrange("(k p) o -> p (k o)", p=P))
        # ones column of V_aug (per-core halo-validity mask)
        for kt_i in range(NKT):
            nc.vector.tensor_copy(
                out=v_sb[:, kt_i, :, 64],
                in_=_brd2(vone_sb[:, kt_i:kt_i + 1], H))

        with tc.tile_pool(name="xtp", bufs=1) as xtp, \
             tc.tile_pool(name="proj", bufs=2, space="PSUM") as projp, \
             tc.tile_pool(name="raw", bufs=2) as rawp, \
             tc.tile_pool(name="mro", bufs=1) as mrp, \
             tc.tile_pool(name="wqk", bufs=1) as wqkp, \
             tc.tile_pool(name="pt", bufs=2) as ptp, \
             tc.tile_pool(name="st", bufs=2, space="PSUM") as stp, \
             tc.tile_pool(name="pv", bufs=2, space="PSUM") as pvp, \
             tc.tile_pool(name="osb", bufs=2) as osbp, \
             tc.tile_pool(name="eps", bufs=1) as epsp:
            xt_sb = xtp.tile([P, 8, SK], bf16)
            nc.sync.dma_start(out=xt_sb,
                              in_=xt.ap().rearrange("(d p) s -> p d s", p=P))

            wk_sb = wqkp.tile([P, 8, DIM], bf16)
            wq_sb = wqkp.tile([P, 8, DIM], bf16)
            wo_sb = wqkp.tile([P, 8, DIM], bf16)
            nc.sync.dma_start(
                out=wk_sb, in_=wk.ap().rearrange("(d p) e -> p d e", p=P))
            nc.sync.dma_start(
                out=wq_sb, in_=wq.ap().rearrange("(d p) e -> p d e", p=P))
            nc.sync.dma_start(
                out=wo_sb, in_=wo.ap().rearrange("(e p) n -> p e n", p=P))

            def rope(raw, dst, nsc, c0):
                # raw [P, 1, nsc] bf16 -> dst (rope'd).  Tables are
                # 32-row-periodic, so shifted reads use the same base for
                # both operands; writes are partition-shifted.
                mcos = mrp.tile([P, nsc], bf16, name="mcos", tag="mcos")
                msw = mrp.tile([P, nsc], bf16, name="msw", tag="msw")
                cseg = slice(c0, c0 + nsc)
                nc.vector.tensor_mul(mcos, raw, ropc_sb[:, cseg])
                for hb in range(2):
                    E = slice(hb * 64, hb * 64 + 32)
                    O = slice(hb * 64 + 32, hb * 64 + 64)
                    nc.vector.tensor_mul(          # O*sin placed at E rows
                        msw[E], raw[O], rops_sb[O, cseg])
                    nc.vector.tensor_mul(          # E*sin placed at O rows
                        msw[O], raw[E], rops_sb[E, cseg])
                for hb in range(2):
                    E = slice(hb * 64, hb * 64 + 32)
                    O = slice(hb * 64 + 32, hb * 64 + 64)
                    nc.vector.tensor_sub(dst[E], mcos[E], msw[E])
                    nc.vector.tensor_add(dst[O], msw[O], mcos[O])

            def proj_rope_et(w_sb, dst, s0, et):
                # dst[:, et, :] = rope((W^T)^T @ xT[:, s0:SK])
                nsc = SK - s0
                raw = rawp.tile([P, nsc], bf16, name="raw", tag="raw")
                for si, sp in enumerate(range(s0, SK, 512)):
                    ps = projp.tile([P, 512], f32, name="ps", tag="ps")
                    for d in range(8):
                        nc.tensor.matmul(
                            ps,
                            lhsT=w_sb[:, d, et * P:(et + 1) * P],
                            rhs=xt_sb[:, d, sp:sp + 512],
                            start=(d == 0), stop=(d == 7))
                    nc.scalar.copy(out=raw[:, si * 512:(si + 1) * 512],
                                   in_=ps)
                rope(raw, dst[:, et, :], nsc, s0)

            def attention_head(h):
                et, hr = h // 2, (h % 2) * 64
                pv_t = [None, None]
                for kt in range(NKT):
                    lo, hi = max(kt - 4, 0), min(kt, 7)
                    nqb = hi - lo + 1
                    kh = kt_sb[hr:hr + 64, et, kt * P:(kt + 1) * P]
                    # scores S^T[k, q] for q blocks lo..hi in one 2-bank psum
                    st_ps = stp.tile([P, 640], f32, name="st_ps")
                    mms = []
                    n0 = min(nqb, 4) * P
                    mms.append((kh, qt_sb[hr:hr + 64, et, lo * P:lo * P + n0],
                                slice(0, n0), True))
                    if nqb == 5:
                        mms.append((kh,
                                    qt_sb[hr:hr + 64, et,
                                          (lo + 4) * P:(lo + 5) * P],
                                    slice(512, 640), True))
                    if kt >= 4:                       # diag mask @ col 0
                        mms.append((udia_sb, negi_sb, slice(0, P), False))
                    if kt <= 7:                       # oldest mask @ col kt-lo
                        c = (kt - lo) * P
                        mms.append((uold_sb, negi_sb, slice(c, c + P), False))
                    for lhsT, rhs, csl, is_start in mms:
                        nc.tensor.matmul(
                            st_ps[:, csl], lhsT=lhsT, rhs=rhs,
                            start=is_start, stop=not is_start,
                            skip_group_check=True)
                    p_t = ptp.tile([P, 640], bf16, name="p_t")
                    nc.scalar.activation(
                        out=p_t[:, 0:nqb * P], in_=st_ps[:, 0:nqb * P],
                        func=mybir.ActivationFunctionType.Exp, scale=0.125)
                    # PV: one matmul per touched pv bank, batched over qbs.
                    # start=True clears the WHOLE psum bank, so only the very
                    # first matmul into each bank sets it; later groups\'
                    # first writes overwrite via has_written=0.
                    for g in (0, 1):
                        c0, c1 = max(lo, 4 * g), min(hi, 4 * g + 3)
                        if c0 > c1:
                            continue
                        if pv_t[g] is None:
                            pv_t[g] = pvp.tile([P, 512], f32, name="pvt",
                                               tag="pvt")
                        nc.tensor.matmul(
                            pv_t[g][0:VA, (c0 % 4) * P:(c1 % 4 + 1) * P],
                            lhsT=v_sb[:, kt, h, :],
                            rhs=p_t[:, (c0 - lo) * P:(c1 - lo + 1) * P],
                            start=(kt == 4 * g), stop=(kt == 4 * g + 7),
                            skip_group_check=True)
                    for g in (0, 1):
                        if kt == 4 * g + 7:
                            # retire group: normalize by ones-row denominator
                            rc = epsp.tile([1, 512], f32, name="rc", tag="rc")
                            bc = epsp.tile([64, 512], f32, name="bc",
                                           tag="bc")
                            rcs = epsp.tile([128, 4], f32, name="rcs",
                                            tag="rcs")
                            rcr = epsp.tile([128, 4], f32, name="rcr",
                                            tag="rcr")
                            # reshape the 512 denominators across all 128
                            # DVE lanes (reciprocal is ~6.5 cyc/free-elem on
                            # one lane otherwise), then shape back
                            rrow = epsp.tile([1, 512], f32, name="rrow",
                                             tag="rrow")
                            nc.scalar.copy(out=rrow, in_=pv_t[g][64:65, :])
                            nc.sync.dma_start(out=rcs, in_=rrow)
                            nc.vector.reciprocal(rcr, rcs)
                            nc.sync.dma_start(out=rc, in_=rcr)
                            nc.gpsimd.partition_broadcast(bc, rc)
                            nc.vector.tensor_mul(
                                ot_sb[hr:hr + 64, et, g * 512:(g + 1) * 512],
                                pv_t[g][0:64, :], bc)

            # ---- main loop: pipelined two etiles ahead so the PE never
            # waits on the DVE rope chain ----
            proj_rope_et(wk_sb, kt_sb, 0, 0)
            proj_rope_et(wq_sb, qt_sb, W, 0)
            proj_rope_et(wk_sb, kt_sb, 0, 1)
            proj_rope_et(wq_sb, qt_sb, W, 1)

            # ---- V projection (emitted after 2 etiles of K/Q proj so
            # the scalar/vector engines have rope+exp work during it) ----
            with tc.tile_pool(name="wvp", bufs=1) as wvp:
                wv_sb = wvp.tile([P, 8, DIM], bf16)
                nc.sync.dma_start(
                    out=wv_sb, in_=wv.ap().rearrange("(d p) e -> p d e", p=P))
                for st_i in range(NKT):
                    for eh in range(2):
                        ps = projp.tile([P, 512], f32, name="psv", tag="ps")
                        for d in range(8):
                            nc.tensor.matmul(
                                ps,
                                lhsT=xt_sb[:, d, st_i * P:(st_i + 1) * P],
                                rhs=wv_sb[:, d, eh * 512:(eh + 1) * 512],
                                start=(d == 0), stop=(d == 7))
                        # scatter heads into V_aug slots [st, h, 0:64]
                        nc.scalar.copy(
                            out=v_sb[:, st_i, eh * 8:(eh + 1) * 8, 0:64],
                            in_=ps[:, :].rearrange("p (h e) -> p h e", h=8))

            for et in range(8):
                attention_head(2 * et)
                attention_head(2 * et + 1)
                if et + 2 < 8:
                    proj_rope_et(wk_sb, kt_sb, 0, et + 2)
                    proj_rope_et(wq_sb, qt_sb, W, et + 2)

            if dbg_outs is not None:
                nc.sync.dma_start(out=dbg_outs["d_qt"][:, :],
                                  in_=qt_sb[:, :, :])
                nc.sync.dma_start(out=dbg_outs["d_kt"][:, :],
                                  in_=kt_sb[:, :, :])
                nc.sync.dma_start(out=dbg_outs["d_v"][:, :],
                                  in_=v_sb[:, :, :, :])
                nc.sync.dma_start(out=dbg_outs["d_ot"][:, :],
                                  in_=ot_sb[:, :, :])

            # ---- output projection ----
            for qt_i in range(NQB):
                for nh in range(2):
                    ps = projp.tile([P, 512], f32, name="pso", tag="ps")
                    for p in range(8):
                        nc.tensor.matmul(
                            ps,
                            lhsT=ot_sb[:, p, qt_i * P:(qt_i + 1) * P],
                            rhs=wo_sb[:, p, nh * 512:(nh + 1) * 512],
                            start=(p == 0), stop=(p == 7))
                    o_sb = osbp.tile([P, 512], f32, name="o_sb")
                    nc.vector.tensor_copy(o_sb, ps)
                    nc.sync.dma_start(
                        out=out[qt_i * P:(qt_i + 1) * P,
                                nh * 512:(nh + 1) * 512],
                        in_=o_sb)


def _prep_inputs(x, Wq, Wk, Wv, Wo):
    """Host-side shard/layout prep -> list of 8 per-core input dicts."""
    x2 = np.ascontiguousarray(x.reshape(S, DIM).astype(np.float32))
    sigma = np.zeros(DIM, dtype=np.int64)
    for h in range(H):
        j = np.arange(32)
        sigma[h * 64 + j] = h * 64 + 2 * j
        sigma[h * 64 + 32 + j] = h * 64 + 2 * j + 1
    wq_h = np.ascontiguousarray(Wq.T[:, sigma]).astype(BF)
    wk_h = np.ascontiguousarray(Wk.T[:, sigma]).astype(BF)
    wv_h = np.ascontiguousarray(Wv.T).astype(BF)
    wo_h = np.ascontiguousarray(Wo.T).astype(BF)

    jj = np.arange(P)
    uold_h = (jj[None, :] <= jj[:, None]).astype(np.float32).astype(BF)
    udia_h = (jj[None, :] >= jj[:, None] + 1).astype(np.float32).astype(BF)
    negi_h = (-1e6 * np.eye(P, dtype=np.float32)).astype(BF)

    inv_freq = 1.0 / (10000.0 ** (np.arange(0, D, 2, dtype=np.float32) / D))
    xT = x2.T  # [DIM, S]

    in_maps = []
    for core in range(NCORES):
        lo = core * SL - W
        xsh = np.zeros((DIM, SK), dtype=np.float32)
        if lo < 0:
            xsh[:, W:] = xT[:, :SL]
        else:
            xsh[:, :] = xT[:, lo:lo + SK]
        pos = np.arange(lo, lo + SK, dtype=np.float32)
        ang = pos[None, :] * inv_freq[:, None]          # [32, SK]
        in_maps.append({
            "xt": xsh.astype(BF),
            "wq": wq_h, "wk": wk_h, "wv": wv_h, "wo": wo_h,
            "ropc": np.ascontiguousarray(
                np.tile(np.cos(ang), (4, 1))).astype(BF),
            "rops": np.ascontiguousarray(
                np.tile(np.sin(ang), (4, 1))).astype(BF),
            "vone": (pos >= 0).astype(np.float32).astype(BF)[:, None],
            "uold": uold_h, "udia": udia_h, "negi": negi_h,
        })
    return in_maps


def kernel(x, Wq, Wk, Wv, Wo, window_size, _trace=False, _trace_kwargs=None):
    assert int(window_size) == W
    if "nc" not in _compiled:
        _compiled["nc"] = _build()
    nc = _compiled["nc"]
    in_maps = _prep_inputs(np.asarray(x), np.asarray(Wq), np.asarray(Wk),
                           np.asarray(Wv), np.asarray(Wo))
    res = run_bass_kernel_spmd(nc, in_maps, core_ids=list(range(NCORES)),
                               trace=_trace, **(_trace_kwargs or {}))
    outp = np.concatenate([res.results[c]["out"] for c in range(NCORES)],
                          axis=0)
    _compiled["last_result"] = res
    return outp.reshape(1, S, DIM).astype(np.float32)


if __name__ == "__main__":
    np.random.seed(0)
    x = np.random.randn(1, S, DIM).astype(np.float32)
    sd = 1.0 / np.sqrt(DIM)
    ws = [np.random.randn(DIM, DIM).astype(np.float32) * sd for _ in range(4)]
    y = kernel(x, *ws, window_size=W)
    print("kernel output", y.shape, y.dtype, np.abs(y).max())


# revision 17
# speedup vs baseline: 1.1137x; 1.1137x over previous
"""Self-contained Trainium2 Bass kernel for sliding-window attention.

Problem (hardcoded): B=1, S=8192, dim=1024, H=16 heads, D=64 head dim,
window=512, fp32 I/O.  y = (softmax(mask(rope(xWq^T) rope(xWk^T)^T / 8)) xWv^T) Wo^T

Strategy: sequence-parallel over 8 NeuronCores. Each core owns 1024 query
rows and additionally recomputes K/V for the 512-row halo to its left
(core 0's halo is zero-padded and neutralized via a per-core "vones"
column so no collective is needed).  All matmuls run in bf16 (fp32 PSUM
accumulation); sliding-window causality is applied by extra rank-128
mask matmuls accumulated into the score PSUM before the exp.

Layouts (per core):
  xT    [1024(d), 1536(s)]  x^T shard incl. halo (bf16)
  wq/wk [1024(d), 1024(e')] Wq^T / Wk^T with a per-head even/odd column
                            permutation (rope pair de-interleave: head h's
                            rows are [evens(32) | odds(32)])
  wv    [1024(d), 1024(e)]  Wv^T (no permutation), wo = Wo^T
  Q^T/K^T are produced in [e', s] layout (weight-stationary matmuls) so
  attention needs no transposes: scores are computed transposed,
  S^T[k, q], the softmax denominator comes free from a ones-column
  appended to V, and PV directly yields o^T[e, q] — the lhsT of the
  output projection.  Rope runs on the vector engine using 32-periodic
  cos/sin tables (equal-base operand reads + partition-shifted writes).
"""
import sys

sys.path.insert(0, "/opt/trn_rl_repo")

import numpy as np
import ml_dtypes

import concourse.bass as bass
import concourse.mybir as mybir
from concourse import bacc
from concourse.tile import TileContext
from concourse.bass_utils import run_bass_kernel_spmd

BF = ml_dtypes.bfloat16
NCORES = 8
S, DIM, H, D, W = 8192, 1024, 16, 64, 512
SL = S // NCORES          # 1024 own rows / core
SK = SL + W               # 1536 rows incl. left halo
P = 128
NKT = SK // P             # 12 kv tiles
NQB = SL // P             # 8 query tiles
dt = mybir.dt

_compiled = {}


def _build(dbg=False):
    nc = bacc.Bacc("TRN2", target_bir_lowering=False, debug=False,
                   num_devices=NCORES)
    def param(name, shape, dtype=dt.bfloat16, out=False):
        return nc.declare_dram_parameter(name, shape, dtype, isOutput=out)

    xt = param("xt", [DIM, SK])
    wq = param("wq", [DIM, DIM])
    wk = param("wk", [DIM, DIM])
    wv = param("wv", [DIM, DIM])
    wo = param("wo", [DIM, DIM])
    ropc = param("ropc", [P, SK])
    rops = param("rops", [P, SK])
    vone = param("vone", [SK, 1])
    uold = param("uold", [P, P])
    udia = param("udia", [P, P])
    negi = param("negi", [P, P])
    out = param("out", [SL, DIM], dt.float32, out=True)
    dbg_outs = None
    if dbg:
        dbg_outs = {
            "d_qt": param("d_qt", [P, 8 * SL], out=True),
            "d_kt": param("d_kt", [P, 8 * SK], out=True),
            "d_v": param("d_v", [P, NKT * H * 80], out=True),
            "d_ot": param("d_ot", [P, 8 * SL], out=True),
                                }

    with TileContext(nc) as tc:
        _body(nc, tc, xt, wq, wk, wv, wo, ropc, rops, vone, uold, udia,
              negi, out, dbg_outs)
    nc.compile()
    return nc


def _brd2(ap_slice, n):
    """Insert a stride-0 middle free dim of size n into a [p, c] AP."""
    return bass.AP(tensor=ap_slice.tensor, offset=ap_slice.offset,
                   ap=[ap_slice.ap[0], [0, n], ap_slice.ap[1]])


VA = 80   # V_aug columns: 64 V + 1 ones + pad (32B-aligned stride)


def _body(nc, tc, xt, wq, wk, wv, wo, ropc, rops, vone, uold, udia, negi,
          out, dbg_outs=None):
    f32, bf16 = dt.float32, dt.bfloat16
    mult, add = mybir.AluOpType.mult, mybir.AluOpType.add

    with tc.tile_pool(name="persist", bufs=1) as per:
        # long-lived SBUF tensors
        v_sb = per.tile([P, NKT, H, VA], bf16)    # V_aug: [V(64)|ones|pad]
        qt_sb = per.tile([P, 8, SL], bf16)        # Q^T (rope'd, sigma layout)
        kt_sb = per.tile([P, 8, SK], bf16)        # K^T
        ot_sb = per.tile([P, 8, SL], bf16)        # o^T (normalized)
        ropc_sb = per.tile([P, SK], bf16)
        rops_sb = per.tile([P, SK], bf16)
        uold_sb = per.tile([P, P], bf16)
        udia_sb = per.tile([P, P], bf16)
        negi_sb = per.tile([P, P], bf16)
        vone_sb = per.tile([P, NKT], bf16)

        nc.sync.dma_start(out=ropc_sb, in_=ropc[:, :])
        nc.sync.dma_start(out=rops_sb, in_=rops[:, :])
        nc.sync.dma_start(out=uold_sb, in_=uold[:, :])
        nc.sync.dma_start(out=udia_sb, in_=udia[:, :])
        nc.sync.dma_start(out=negi_sb, in_=negi[:, :])
        nc.sync.dma_start(out=vone_sb,
                          in_=vone.ap().rearrange("(k p) o -> p (k o)", p=P))
        # ones column of V_aug (per-core halo-validity mask)
        for kt_i in range(NKT):
            nc.vector.tensor_copy(
                out=v_sb[:, kt_i, :, 64],
                in_=_brd2(vone_sb[:, kt_i:kt_i + 1], H))

        with tc.tile_pool(name="xtp", bufs=1) as xtp, \
             tc.tile_pool(name="proj", bufs=1, space="PSUM") as projp, \
             tc.tile_pool(name="raw", bufs=2) as rawp, \
             tc.tile_pool(name="mro", bufs=1) as mrp, \
             tc.tile_pool(name="wqk", bufs=1) as wqkp, \
             tc.tile_pool(name="pt", bufs=2) as ptp, \
             tc.tile_pool(name="st", bufs=2, space="PSUM") as stp, \
             tc.tile_pool(name="pv", bufs=3, space="PSUM") as pvp, \
             tc.tile_pool(name="osb", bufs=2) as osbp, \
             tc.tile_pool(name="eps", bufs=1) as epsp:
            xt_sb = xtp.tile([P, 8, SK], bf16)
            nc.sync.dma_start(out=xt_sb,
                              in_=xt.ap().rearrange("(d p) s -> p d s", p=P))

            wk_sb = wqkp.tile([P, 8, DIM], bf16)
            wq_sb = wqkp.tile([P, 8, DIM], bf16)
            wo_sb = wqkp.tile([P, 8, DIM], bf16)
            nc.sync.dma_start(
                out=wk_sb, in_=wk.ap().rearrange("(d p) e -> p d e", p=P))
            nc.sync.dma_start(
                out=wq_sb, in_=wq.ap().rearrange("(d p) e -> p d e", p=P))
            nc.sync.dma_start(
                out=wo_sb, in_=wo.ap().rearrange("(e p) n -> p e n", p=P))

            def rope(raw, dst, nsc, c0):
                # raw [P, 1, nsc] bf16 -> dst (rope'd).  Tables are
                # 32-row-periodic, so shifted reads use the same base for
                # both operands; writes are partition-shifted.
                mcos = mrp.tile([P, nsc], bf16, name="mcos", tag="mcos")
                msw = mrp.tile([P, nsc], bf16, name="msw", tag="msw")
                cseg = slice(c0, c0 + nsc)
                nc.vector.tensor_mul(mcos, raw, ropc_sb[:, cseg])
                for hb in range(2):
                    E = slice(hb * 64, hb * 64 + 32)
                    O = slice(hb * 64 + 32, hb * 64 + 64)
                    nc.vector.tensor_mul(          # O*sin placed at E rows
                        msw[E], raw[O], rops_sb[O, cseg])
                    nc.vector.tensor_mul(          # E*sin placed at O rows
                        msw[O], raw[E], rops_sb[E, cseg])
                for hb in range(2):
                    E = slice(hb * 64, hb * 64 + 32)
                    O = slice(hb * 64 + 32, hb * 64 + 64)
                    nc.vector.tensor_sub(dst[E], mcos[E], msw[E])
                    nc.vector.tensor_add(dst[O], msw[O], mcos[O])

            def proj_rope_et(w_sb, dst, s0, et):
                # dst[:, et, :] = rope((W^T)^T @ xT[:, s0:SK])
                nsc = SK - s0
                raw = rawp.tile([P, nsc], bf16, name="raw", tag="raw")
                for si, sp in enumerate(range(s0, SK, 512)):
                    ps = projp.tile([P, 512], f32, name="ps", tag="ps")
                    for d in range(8):
                        nc.tensor.matmul(
                            ps,
                            lhsT=w_sb[:, d, et * P:(et + 1) * P],
                            rhs=xt_sb[:, d, sp:sp + 512],
                            start=(d == 0), stop=(d == 7))
                    nc.scalar.copy(out=raw[:, si * 512:(si + 1) * 512],
                                   in_=ps)
                rope(raw, dst[:, et, :], nsc, s0)

            def attention_head(h):
                et, hr = h // 2, (h % 2) * 64
                pv_t = [None, None]
                for kt in range(NKT):
                    lo, hi = max(kt - 4, 0), min(kt, 7)
                    nqb = hi - lo + 1
                    kh = kt_sb[hr:hr + 64, et, kt * P:(kt + 1) * P]
                    # scores S^T[k, q] for q blocks lo..hi in one 2-bank psum
                    st_ps = stp.tile([P, 640], f32, name="st_ps")
                    mms = []
                    n0 = min(nqb, 4) * P
                    mms.append((kh, qt_sb[hr:hr + 64, et, lo * P:lo * P + n0],
                                slice(0, n0), True))
                    if nqb == 5:
                        mms.append((kh,
                                    qt_sb[hr:hr + 64, et,
                                          (lo + 4) * P:(lo + 5) * P],
                                    slice(512, 640), True))
                    if kt >= 4:                       # diag mask @ col 0
                        mms.append((udia_sb, negi_sb, slice(0, P), False))
                    if kt <= 7:                       # oldest mask @ col kt-lo
                        c = (kt - lo) * P
                        mms.append((uold_sb, negi_sb, slice(c, c + P), False))
                    for lhsT, rhs, csl, is_start in mms:
                        nc.tensor.matmul(
                            st_ps[:, csl], lhsT=lhsT, rhs=rhs,
                            start=is_start, stop=not is_start,
                            skip_group_check=True)
                    p_t = ptp.tile([P, 640], bf16, name="p_t")
                    nc.scalar.activation(
                        out=p_t[:, 0:nqb * P], in_=st_ps[:, 0:nqb * P],
                        func=mybir.ActivationFunctionType.Exp, scale=0.125)
                    # PV: one matmul per touched pv bank, batched over qbs.
                    # start=True clears the WHOLE psum bank, so only the very
                    # first matmul into each bank sets it; later groups\'
                    # first writes overwrite via has_written=0.
                    for g in (0, 1):
                        c0, c1 = max(lo, 4 * g), min(hi, 4 * g + 3)
                        if c0 > c1:
                            continue
                        if pv_t[g] is None:
                            pv_t[g] = pvp.tile([P, 512], f32, name="pvt",
                                               tag="pvt")
                        nc.tensor.matmul(
                            pv_t[g][0:VA, (c0 % 4) * P:(c1 % 4 + 1) * P],
                            lhsT=v_sb[:, kt, h, :],
                            rhs=p_t[:, (c0 - lo) * P:(c1 - lo + 1) * P],
                            start=(kt == 4 * g), stop=(kt == 4 * g + 7),
                            skip_group_check=True)
                    for g in (0, 1):
                        if kt == 4 * g + 7:
                            # retire group: normalize by ones-row denominator
                            rc = epsp.tile([1, 512], f32, name="rc", tag="rc")
                            bc = epsp.tile([64, 512], f32, name="bc",
                                           tag="bc")
                            rcs = epsp.tile([128, 4], f32, name="rcs",
                                            tag="rcs")
                            rcr = epsp.tile([128, 4], f32, name="rcr",
                                            tag="rcr")
                            # reshape the 512 denominators across all 128
                            # DVE lanes (reciprocal is ~6.5 cyc/free-elem on
                            # one lane otherwise), then shape back
                            rrow = epsp.tile([1, 512], f32, name="rrow",
                                             tag="rrow")
                            nc.scalar.copy(out=rrow, in_=pv_t[g][64:65, :])
                            nc.sync.dma_start(out=rcs, in_=rrow)
                            nc.vector.reciprocal(rcr, rcs)
                            nc.sync.dma_start(out=rc, in_=rcr)
                            nc.gpsimd.partition_broadcast(bc, rc)
                            nc.vector.tensor_mul(
                                ot_sb[hr:hr + 64, et, g * 512:(g + 1) * 512],
                                pv_t[g][0:64, :], bc)

            # ---- main loop: pipelined two etiles ahead so the PE never
            # waits on the DVE rope chain ----
            proj_rope_et(wk_sb, kt_sb, 0, 0)
            proj_rope_et(wq_sb, qt_sb, W, 0)
            proj_rope_et(wk_sb, kt_sb, 0, 1)
            proj_rope_et(wq_sb, qt_sb, W, 1)

            # ---- V projection (emitted after 2 etiles of K/Q proj so
            # the scalar/vector engines have rope+exp work during it) ----
            with tc.tile_pool(name="wvp", bufs=1) as wvp:
                wv_sb = wvp.tile([P, 8, DIM], bf16)
                nc.sync.dma_start(
                    out=wv_sb, in_=wv.ap().rearrange("(d p) e -> p d e", p=P))
                for st_i in range(NKT):
                    for eh in range(2):
                        ps = projp.tile([P, 512], f32, name="psv", tag="ps")
                        for d in range(8):
                            nc.tensor.matmul(
                                ps,
                                lhsT=xt_sb[:, d, st_i * P:(st_i + 1) * P],
                                rhs=wv_sb[:, d, eh * 512:(eh + 1) * 512],
                                start=(d == 0), stop=(d == 7))
                        # scatter heads into V_aug slots [st, h, 0:64]
                        nc.scalar.copy(
                            out=v_sb[:, st_i, eh * 8:(eh + 1) * 8, 0:64],
                            in_=ps[:, :].rearrange("p (h e) -> p h e", h=8))

            for et in range(8):
                attention_head(2 * et)
                attention_head(2 * et + 1)
                if et + 2 < 8:
                    proj_rope_et(wk_sb, kt_sb, 0, et + 2)
                    proj_rope_et(wq_sb, qt_sb, W, et + 2)

            if dbg_outs is not None:
                nc.sync.dma_start(out=dbg_outs["d_qt"][:, :],
                                  in_=qt_sb[:, :, :])
                nc.sync.dma_start(out=dbg_outs["d_kt"][:, :],
                                  in_=kt_sb[:, :, :])
                nc.sync.dma_start(out=dbg_outs["d_v"][:, :],
                                  in_=v_sb[:, :, :, :])
                nc.sync.dma_start(out=dbg_outs["d_ot"][:, :],
                                  in_=ot_sb[:, :, :])

            # ---- output projection ----
            for qt_i in range(NQB):
                for nh in range(2):
                    ps = projp.tile([P, 512], f32, name="pso", tag="ps")
                    for p in range(8):
                        nc.tensor.matmul(
                            ps,
                            lhsT=ot_sb[:, p, qt_i * P:(qt_i + 1) * P],
                            rhs=wo_sb[:, p, nh * 512:(nh + 1) * 512],
                            start=(p == 0), stop=(p == 7))
                    o_sb = osbp.tile([P, 512], f32, name="o_sb")
                    nc.vector.tensor_copy(o_sb, ps)
                    nc.sync.dma_start(
                        out=out[qt_i * P:(qt_i + 1) * P,
                                nh * 512:(nh + 1) * 512],
                        in_=o_sb)


def _prep_inputs(x, Wq, Wk, Wv, Wo):
    """Host-side shard/layout prep -> list of 8 per-core input dicts."""
    x2 = np.ascontiguousarray(x.reshape(S, DIM).astype(np.float32))
    sigma = np.zeros(DIM, dtype=np.int64)
    for h in range(H):
        j = np.arange(32)
        sigma[h * 64 + j] = h * 64 + 2 * j
        sigma[h * 64 + 32 + j] = h * 64 + 2 * j + 1
    wq_h = np.ascontiguousarray(Wq.T[:, sigma]).astype(BF)
    wk_h = np.ascontiguousarray(Wk.T[:, sigma]).astype(BF)
    wv_h = np.ascontiguousarray(Wv.T).astype(BF)
    wo_h = np.ascontiguousarray(Wo.T).astype(BF)

    jj = np.arange(P)
    uold_h = (jj[None, :] <= jj[:, None]).astype(np.float32).astype(BF)
    udia_h = (jj[None, :] >= jj[:, None] + 1).astype(np.float32).astype(BF)
    negi_h = (-1e6 * np.eye(P, dtype=np.float32)).astype(BF)

    inv_freq = 1.0 / (10000.0 ** (np.arange(0, D, 2, dtype=np.float32) / D))
    xT = x2.T  # [DIM, S]

    in_maps = []
    for core in range(NCORES):
        lo = core * SL - W
        xsh = np.zeros((DIM, SK), dtype=np.float32)
        if lo < 0:
            xsh[:, W:] = xT[:, :SL]
        else:
            xsh[:, :] = xT[:, lo:lo + SK]
        pos = np.arange(lo, lo + SK, dtype=np.float32)
        ang = pos[None, :] * inv_freq[:, None]          # [32, SK]
        in_maps.append({
            "xt": xsh.astype(BF),
            "wq": wq_h, "wk": wk_h, "wv": wv_h, "wo": wo_h,
            "ropc": np.ascontiguousarray(
                np.tile(np.cos(ang), (4, 1))).astype(BF),
            "rops": np.ascontiguousarray(
                np.tile(np.sin(ang), (4, 1))).astype(BF),
            "vone": (pos >= 0).astype(np.float32).astype(BF)[:, None],
            "uold": uold_h, "udia": udia_h, "negi": negi_h,
        })
    return in_maps


def kernel(x, Wq, Wk, Wv, Wo, window_size, _trace=False, _trace_kwargs=None):
    assert int(window_size) == W
    if "nc" not in _compiled:
        _compiled["nc"] = _build()
    nc = _compiled["nc"]
    in_maps = _prep_inputs(np.asarray(x), np.asarray(Wq), np.asarray(Wk),
                           np.asarray(Wv), np.asarray(Wo))
    res = run_bass_kernel_spmd(nc, in_maps, core_ids=list(range(NCORES)),
                               trace=_trace, **(_trace_kwargs or {}))
    outp = np.concatenate([res.results[c]["out"] for c in range(NCORES)],
                          axis=0)
    _compiled["last_result"] = res
    return outp.reshape(1, S, DIM).astype(np.float32)


if __name__ == "__main__":
    np.random.seed(0)
    x = np.random.randn(1, S, DIM).astype(np.float32)
    sd = 1.0 / np.sqrt(DIM)
    ws = [np.random.randn(DIM, DIM).astype(np.float32) * sd for _ in range(4)]
    y = kernel(x, *ws, window_size=W)
    print("kernel output", y.shape, y.dtype, np.abs(y).max())


# revision 18
# speedup vs baseline: 1.1366x; 1.0206x over previous
"""Self-contained Trainium2 Bass kernel for sliding-window attention.

Problem (hardcoded): B=1, S=8192, dim=1024, H=16 heads, D=64 head dim,
window=512, fp32 I/O.  y = (softmax(mask(rope(xWq^T) rope(xWk^T)^T / 8)) xWv^T) Wo^T

Strategy: sequence-parallel over 8 NeuronCores. Each core owns 1024 query
rows and additionally recomputes K/V for the 512-row halo to its left
(core 0's halo is zero-padded and neutralized via a per-core "vones"
column so no collective is needed).  All matmuls run in bf16 (fp32 PSUM
accumulation); sliding-window causality is applied by extra rank-128
mask matmuls accumulated into the score PSUM before the exp.

Layouts (per core):
  xT    [1024(d), 1536(s)]  x^T shard incl. halo (bf16)
  wq/wk [1024(d), 1024(e')] Wq^T / Wk^T with a per-head even/odd column
                            permutation (rope pair de-interleave: head h's
                            rows are [evens(32) | odds(32)])
  wv    [1024(d), 1024(e)]  Wv^T (no permutation), wo = Wo^T
  Q^T/K^T are produced in [e', s] layout (weight-stationary matmuls) so
  attention needs no transposes: scores are computed transposed,
  S^T[k, q], the softmax denominator comes free from a ones-column
  appended to V, and PV directly yields o^T[e, q] — the lhsT of the
  output projection.  Rope runs on the vector engine using 32-periodic
  cos/sin tables (equal-base operand reads + partition-shifted writes).
"""
import sys

sys.path.insert(0, "/opt/trn_rl_repo")

import numpy as np
import ml_dtypes

import concourse.bass as bass
import concourse.mybir as mybir
from concourse import bacc
from concourse.tile import TileContext
from concourse.bass_utils import run_bass_kernel_spmd

BF = ml_dtypes.bfloat16
NCORES = 8
S, DIM, H, D, W = 8192, 1024, 16, 64, 512
SL = S // NCORES          # 1024 own rows / core
SK = SL + W               # 1536 rows incl. left halo
P = 128
NKT = SK // P             # 12 kv tiles
NQB = SL // P             # 8 query tiles
dt = mybir.dt

_compiled = {}


def _build(dbg=False):
    nc = bacc.Bacc("TRN2", target_bir_lowering=False, debug=False,
                   num_devices=NCORES)
    def param(name, shape, dtype=dt.bfloat16, out=False):
        return nc.declare_dram_parameter(name, shape, dtype, isOutput=out)

    xt = param("xt", [DIM, SK])
    wq = param("wq", [DIM, DIM])
    wk = param("wk", [DIM, DIM])
    wv = param("wv", [DIM, DIM])
    wo = param("wo", [DIM, DIM])
    ropc = param("ropc", [P, SK])
    rops = param("rops", [P, SK])
    vone = param("vone", [SK, 1])
    uold = param("uold", [P, P])
    udia = param("udia", [P, P])
    negi = param("negi", [P, P])
    out = param("out", [SL, DIM], dt.float32, out=True)
    dbg_outs = None
    if dbg:
        dbg_outs = {
            "d_qt": param("d_qt", [P, 8 * SL], out=True),
            "d_kt": param("d_kt", [P, 8 * SK], out=True),
            "d_v": param("d_v", [P, NKT * H * 80], out=True),
            "d_ot": param("d_ot", [P, 8 * SL], out=True),
                                }

    with TileContext(nc) as tc:
        _body(nc, tc, xt, wq, wk, wv, wo, ropc, rops, vone, uold, udia,
              negi, out, dbg_outs)
    nc.compile()
    return nc


def _brd2(ap_slice, n):
    """Insert a stride-0 middle free dim of size n into a [p, c] AP."""
    return bass.AP(tensor=ap_slice.tensor, offset=ap_slice.offset,
                   ap=[ap_slice.ap[0], [0, n], ap_slice.ap[1]])


VA = 80   # V_aug columns: 64 V + 1 ones + pad (32B-aligned stride)


def _body(nc, tc, xt, wq, wk, wv, wo, ropc, rops, vone, uold, udia, negi,
          out, dbg_outs=None):
    f32, bf16 = dt.float32, dt.bfloat16
    mult, add = mybir.AluOpType.mult, mybir.AluOpType.add

    with tc.tile_pool(name="persist", bufs=1) as per:
        # long-lived SBUF tensors
        v_sb = per.tile([P, NKT, H, VA], bf16)    # V_aug: [V(64)|ones|pad]
        qt_sb = per.tile([P, 8, SL], bf16)        # Q^T (rope'd, sigma layout)
        kt_sb = per.tile([P, 8, SK], bf16)        # K^T
        ot_sb = per.tile([P, 8, SL], bf16)        # o^T (normalized)
        ropc_sb = per.tile([P, SK], bf16)
        rops_sb = per.tile([P, SK], bf16)
        uold_sb = per.tile([P, P], bf16)
        udia_sb = per.tile([P, P], bf16)
        negi_sb = per.tile([P, P], bf16)
        vone_sb = per.tile([P, NKT], bf16)

        nc.sync.dma_start(out=ropc_sb, in_=ropc[:, :])
        nc.sync.dma_start(out=rops_sb, in_=rops[:, :])
        nc.sync.dma_start(out=uold_sb, in_=uold[:, :])
        nc.sync.dma_start(out=udia_sb, in_=udia[:, :])
        nc.sync.dma_start(out=negi_sb, in_=negi[:, :])
        nc.sync.dma_start(out=vone_sb,
                          in_=vone.ap().rearrange("(k p) o -> p (k o)", p=P))
        # ones column of V_aug (per-core halo-validity mask)
        for kt_i in range(NKT):
            nc.vector.tensor_copy(
                out=v_sb[:, kt_i, :, 64],
                in_=_brd2(vone_sb[:, kt_i:kt_i + 1], H))

        with tc.tile_pool(name="xtp", bufs=1) as xtp, \
             tc.tile_pool(name="proj", bufs=1, space="PSUM") as projp, \
             tc.tile_pool(name="raw", bufs=2) as rawp, \
             tc.tile_pool(name="mro", bufs=1) as mrp, \
             tc.tile_pool(name="wqk", bufs=1) as wqkp, \
             tc.tile_pool(name="pt", bufs=2) as ptp, \
             tc.tile_pool(name="st", bufs=2, space="PSUM") as stp, \
             tc.tile_pool(name="pv", bufs=3, space="PSUM") as pvp, \
             tc.tile_pool(name="osb", bufs=2) as osbp, \
             tc.tile_pool(name="eps", bufs=1) as epsp:
            xt_sb = xtp.tile([P, 8, SK], bf16)
            xt_r = xt.ap().rearrange("(d p) s -> p d s", p=P)
            for d in range(8):
                nc.sync.dma_start(out=xt_sb[:, d, :], in_=xt_r[:, d, :])

            wk_sb = wqkp.tile([P, 8, DIM], bf16)
            wq_sb = wqkp.tile([P, 8, DIM], bf16)
            wo_sb = wqkp.tile([P, 8, DIM], bf16)
            wk_r = wk.ap().rearrange("(d p) e -> p d e", p=P)
            wq_r = wq.ap().rearrange("(d p) e -> p d e", p=P)
            for d in range(8):
                nc.sync.dma_start(out=wk_sb[:, d, :], in_=wk_r[:, d, :])
            for d in range(8):
                nc.sync.dma_start(out=wq_sb[:, d, :], in_=wq_r[:, d, :])
            nc.sync.dma_start(
                out=wo_sb, in_=wo.ap().rearrange("(e p) n -> p e n", p=P))

            def rope(raw, dst, nsc, c0):
                # raw [P, 1, nsc] bf16 -> dst (rope'd).  Tables are
                # 32-row-periodic, so shifted reads use the same base for
                # both operands; writes are partition-shifted.
                mcos = mrp.tile([P, nsc], bf16, name="mcos", tag="mcos")
                msw = mrp.tile([P, nsc], bf16, name="msw", tag="msw")
                cseg = slice(c0, c0 + nsc)
                nc.vector.tensor_mul(mcos, raw, ropc_sb[:, cseg])
                for hb in range(2):
                    E = slice(hb * 64, hb * 64 + 32)
                    O = slice(hb * 64 + 32, hb * 64 + 64)
                    nc.vector.tensor_mul(          # O*sin placed at E rows
                        msw[E], raw[O], rops_sb[O, cseg])
                    nc.vector.tensor_mul(          # E*sin placed at O rows
                        msw[O], raw[E], rops_sb[E, cseg])
                for hb in range(2):
                    E = slice(hb * 64, hb * 64 + 32)
                    O = slice(hb * 64 + 32, hb * 64 + 64)
                    nc.vector.tensor_sub(dst[E], mcos[E], msw[E])
                    nc.vector.tensor_add(dst[O], msw[O], mcos[O])

            def proj_rope_et(w_sb, dst, s0, et):
                # dst[:, et, :] = rope((W^T)^T @ xT[:, s0:SK])
                nsc = SK - s0
                raw = rawp.tile([P, nsc], bf16, name="raw", tag="raw")
                for si, sp in enumerate(range(s0, SK, 512)):
                    ps = projp.tile([P, 512], f32, name="ps", tag="ps")
                    for d in range(8):
                        nc.tensor.matmul(
                            ps,
                            lhsT=w_sb[:, d, et * P:(et + 1) * P],
                            rhs=xt_sb[:, d, sp:sp + 512],
                            start=(d == 0), stop=(d == 7))
                    nc.scalar.copy(out=raw[:, si * 512:(si + 1) * 512],
                                   in_=ps)
                rope(raw, dst[:, et, :], nsc, s0)

            def attention_head(h):
                et, hr = h // 2, (h % 2) * 64
                pv_t = [None, None]
                for kt in range(NKT):
                    lo, hi = max(kt - 4, 0), min(kt, 7)
                    nqb = hi - lo + 1
                    kh = kt_sb[hr:hr + 64, et, kt * P:(kt + 1) * P]
                    # scores S^T[k, q] for q blocks lo..hi in one 2-bank psum
                    st_ps = stp.tile([P, 640], f32, name="st_ps")
                    mms = []
                    n0 = min(nqb, 4) * P
                    mms.append((kh, qt_sb[hr:hr + 64, et, lo * P:lo * P + n0],
                                slice(0, n0), True))
                    if nqb == 5:
                        mms.append((kh,
                                    qt_sb[hr:hr + 64, et,
                                          (lo + 4) * P:(lo + 5) * P],
                                    slice(512, 640), True))
                    if kt >= 4:                       # diag mask @ col 0
                        mms.append((udia_sb, negi_sb, slice(0, P), False))
                    if kt <= 7:                       # oldest mask @ col kt-lo
                        c = (kt - lo) * P
                        mms.append((uold_sb, negi_sb, slice(c, c + P), False))
                    for lhsT, rhs, csl, is_start in mms:
                        nc.tensor.matmul(
                            st_ps[:, csl], lhsT=lhsT, rhs=rhs,
                            start=is_start, stop=not is_start,
                            skip_group_check=True)
                    p_t = ptp.tile([P, 640], bf16, name="p_t")
                    nc.scalar.activation(
                        out=p_t[:, 0:nqb * P], in_=st_ps[:, 0:nqb * P],
                        func=mybir.ActivationFunctionType.Exp, scale=0.125)
                    # PV: one matmul per touched pv bank, batched over qbs.
                    # start=True clears the WHOLE psum bank, so only the very
                    # first matmul into each bank sets it; later groups\'
                    # first writes overwrite via has_written=0.
                    for g in (0, 1):
                        c0, c1 = max(lo, 4 * g), min(hi, 4 * g + 3)
                        if c0 > c1:
                            continue
                        if pv_t[g] is None:
                            pv_t[g] = pvp.tile([P, 512], f32, name="pvt",
                                               tag="pvt")
                        nc.tensor.matmul(
                            pv_t[g][0:VA, (c0 % 4) * P:(c1 % 4 + 1) * P],
                            lhsT=v_sb[:, kt, h, :],
                            rhs=p_t[:, (c0 - lo) * P:(c1 - lo + 1) * P],
                            start=(kt == 4 * g), stop=(kt == 4 * g + 7),
                            skip_group_check=True)
                    for g in (0, 1):
                        if kt == 4 * g + 7:
                            # retire group: normalize by ones-row denominator
                            rc = epsp.tile([1, 512], f32, name="rc", tag="rc")
                            bc = epsp.tile([64, 512], f32, name="bc",
                                           tag="bc")
                            rcs = epsp.tile([128, 4], f32, name="rcs",
                                            tag="rcs")
                            rcr = epsp.tile([128, 4], f32, name="rcr",
                                            tag="rcr")
                            # reshape the 512 denominators across all 128
                            # DVE lanes (reciprocal is ~6.5 cyc/free-elem on
                            # one lane otherwise), then shape back
                            rrow = epsp.tile([1, 512], f32, name="rrow",
                                             tag="rrow")
                            nc.scalar.copy(out=rrow, in_=pv_t[g][64:65, :])
                            nc.sync.dma_start(out=rcs, in_=rrow)
                            nc.vector.reciprocal(rcr, rcs)
                            nc.sync.dma_start(out=rc, in_=rcr)
                            nc.gpsimd.partition_broadcast(bc, rc)
                            nc.vector.tensor_mul(
                                ot_sb[hr:hr + 64, et, g * 512:(g + 1) * 512],
                                pv_t[g][0:64, :], bc)

            # ---- main loop: pipelined two etiles ahead so the PE never
            # waits on the DVE rope chain ----
            proj_rope_et(wk_sb, kt_sb, 0, 0)
            proj_rope_et(wq_sb, qt_sb, W, 0)
            proj_rope_et(wk_sb, kt_sb, 0, 1)
            proj_rope_et(wq_sb, qt_sb, W, 1)

            # ---- V projection (emitted after 2 etiles of K/Q proj so
            # the scalar/vector engines have rope+exp work during it) ----
            with tc.tile_pool(name="wvp", bufs=1) as wvp:
                wv_sb = wvp.tile([P, 8, DIM], bf16)
                nc.sync.dma_start(
                    out=wv_sb, in_=wv.ap().rearrange("(d p) e -> p d e", p=P))
                for st_i in range(NKT):
                    for eh in range(2):
                        ps = projp.tile([P, 512], f32, name="psv", tag="ps")
                        for d in range(8):
                            nc.tensor.matmul(
                                ps,
                                lhsT=xt_sb[:, d, st_i * P:(st_i + 1) * P],
                                rhs=wv_sb[:, d, eh * 512:(eh + 1) * 512],
                                start=(d == 0), stop=(d == 7))
                        # scatter heads into V_aug slots [st, h, 0:64]
                        nc.scalar.copy(
                            out=v_sb[:, st_i, eh * 8:(eh + 1) * 8, 0:64],
                            in_=ps[:, :].rearrange("p (h e) -> p h e", h=8))

            for et in range(8):
                attention_head(2 * et)
                attention_head(2 * et + 1)
                if et + 2 < 8:
                    proj_rope_et(wk_sb, kt_sb, 0, et + 2)
                    proj_rope_et(wq_sb, qt_sb, W, et + 2)

            if dbg_outs is not None:
                nc.sync.dma_start(out=dbg_outs["d_qt"][:, :],
                                  in_=qt_sb[:, :, :])
                nc.sync.dma_start(out=dbg_outs["d_kt"][:, :],
                                  in_=kt_sb[:, :, :])
                nc.sync.dma_start(out=dbg_outs["d_v"][:, :],
                                  in_=v_sb[:, :, :, :])
                nc.sync.dma_start(out=dbg_outs["d_ot"][:, :],
                                  in_=ot_sb[:, :, :])

            # ---- output projection ----
            for qt_i in range(NQB):
                for nh in range(2):
                    ps = projp.tile([P, 512], f32, name="pso", tag="ps")
                    for p in range(8):
                        nc.tensor.matmul(
                            ps,
                            lhsT=ot_sb[:, p, qt_i * P:(qt_i + 1) * P],
                            rhs=wo_sb[:, p, nh * 512:(nh + 1) * 512],
                            start=(p == 0), stop=(p == 7))
                    o_sb = osbp.tile([P, 512], f32, name="o_sb")
                    nc.vector.tensor_copy(o_sb, ps)
                    nc.sync.dma_start(
                        out=out[qt_i * P:(qt_i + 1) * P,
                                nh * 512:(nh + 1) * 512],
                        in_=o_sb)


def _prep_inputs(x, Wq, Wk, Wv, Wo):
    """Host-side shard/layout prep -> list of 8 per-core input dicts."""
    x2 = np.ascontiguousarray(x.reshape(S, DIM).astype(np.float32))
    sigma = np.zeros(DIM, dtype=np.int64)
    for h in range(H):
        j = np.arange(32)
        sigma[h * 64 + j] = h * 64 + 2 * j
        sigma[h * 64 + 32 + j] = h * 64 + 2 * j + 1
    wq_h = np.ascontiguousarray(Wq.T[:, sigma]).astype(BF)
    wk_h = np.ascontiguousarray(Wk.T[:, sigma]).astype(BF)
    wv_h = np.ascontiguousarray(Wv.T).astype(BF)
    wo_h = np.ascontiguousarray(Wo.T).astype(BF)

    jj = np.arange(P)
    uold_h = (jj[None, :] <= jj[:, None]).astype(np.float32).astype(BF)
    udia_h = (jj[None, :] >= jj[:, None] + 1).astype(np.float32).astype(BF)
    negi_h = (-1e6 * np.eye(P, dtype=np.float32)).astype(BF)

    inv_freq = 1.0 / (10000.0 ** (np.arange(0, D, 2, dtype=np.float32) / D))
    xT = x2.T  # [DIM, S]

    in_maps = []
    for core in range(NCORES):
        lo = core * SL - W
        xsh = np.zeros((DIM, SK), dtype=np.float32)
        if lo < 0:
            xsh[:, W:] = xT[:, :SL]
        else:
            xsh[:, :] = xT[:, lo:lo + SK]
        pos = np.arange(lo, lo + SK, dtype=np.float32)
        ang = pos[None, :] * inv_freq[:, None]          # [32, SK]
        in_maps.append({
            "xt": xsh.astype(BF),
            "wq": wq_h, "wk": wk_h, "wv": wv_h, "wo": wo_h,
            "ropc": np.ascontiguousarray(
                np.tile(np.cos(ang), (4, 1))).astype(BF),
            "rops": np.ascontiguousarray(
                np.tile(np.sin(ang), (4, 1))).astype(BF),
            "vone": (pos >= 0).astype(np.float32).astype(BF)[:, None],
            "uold": uold_h, "udia": udia_h, "negi": negi_h,
        })
    return in_maps


def kernel(x, Wq, Wk, Wv, Wo, window_size, _trace=False, _trace_kwargs=None):
    assert int(window_size) == W
    if "nc" not in _compiled:
        _compiled["nc"] = _build()
    nc = _compiled["nc"]
    in_maps = _prep_inputs(np.asarray(x), np.asarray(Wq), np.asarray(Wk),
                           np.asarray(Wv), np.asarray(Wo))
    res = run_bass_kernel_spmd(nc, in_maps, core_ids=list(range(NCORES)),
                               trace=_trace, **(_trace_kwargs or {}))
    outp = np.concatenate([res.results[c]["out"] for c in range(NCORES)],
                          axis=0)
    _compiled["last_result"] = res
    return outp.reshape(1, S, DIM).astype(np.float32)


if __name__ == "__main__":
    np.random.seed(0)
    x = np.random.randn(1, S, DIM).astype(np.float32)
    sd = 1.0 / np.sqrt(DIM)
    ws = [np.random.randn(DIM, DIM).astype(np.float32) * sd for _ in range(4)]
    y = kernel(x, *ws, window_size=W)
    print("kernel output", y.shape, y.dtype, np.abs(y).max())
